# revision 31
# baseline (speedup 1.0000x reference)
"""Causal self-attention (B=4, T=2048, C=1024, H=16, D=64) on 8 TRN2 NeuronCores.

Sharding: core i handles batch b = i//2 and head-group g = i%2 (8 of the 16
heads).  Each core computes the QKV projection for its batch restricted to its
heads' columns, runs causal attention for its 8 heads, and produces a partial
output projection y_part = ctx_g @ w_out[rows of g].  The two partials per
batch are summed on the host (y[b] = y_part[2b] + y_part[2b+1]).

The kernel is PE-cycle-bound (the chip power-throttles the PE clock to ~50%
duty under sustained 8-core matmul load), so the layout minimizes PE work,
and the attention inner loop is ACT(exp)-latency-coupled, so exp-independent
matmuls are interleaved as backlog to keep the in-order PE queue fed:
  - x and w_qkv are cast to bf16 on the host (halves the DMA-bound head;
    total rel err ~0.005 vs the 0.02 budget).
  - q,k are produced transposed ([d, t]); v is produced directly in
    [t, ch] layout (stationary = x chunks), no PE transposes.  Per
    (chunk, head-pair) v is stored as [v_A | ones | v_B]; head A's PV
    stationary [v_A|ones] yields ctx in PSUM rows 0-63 and the softmax
    denominator in rows 64-127, head B's [ones|v_B] the reverse.
  - QK^T for a pair of heads is issued as row-tiled matmuls (head A in
    array rows 0-63 / tile_position (0,0), head B in rows 64-127 /
    (64,0), separate PSUM banks).
  - causal mask applied AFTER exp by zeroing the upper triangle of the
    diagonal 128-block with gpsimd affine_select (raw scores are |s|<~10
    so exp before masking is safe).
  - PV accumulates exact causal ranges (no zero-padding matmuls), one
    512-wide piece at a time, deferred two chunks behind the score
    matmuls so it never waits on a fresh exp.
  - only head-pair 0's q,k projections run before attention starts; all
    other QKV work + the previous block's output projection are pumped
    as per-pair backlog between score chunks.  A backlog closure that a
    later PE instruction depends on is placed at least one pair early
    (the in-order PE queue deadlocks otherwise), and the x pool holds
    all NT n-tiles so no x-DMA WAR can cycle through the backlog.
"""

from functools import partial

import numpy as np
import ml_dtypes

import concourse.bass as bass
import concourse.mybir as mybir
from concourse import bacc, tile
from concourse.bass_utils import run_bass_kernel_spmd

F32 = mybir.dt.float32
BF16 = mybir.dt.bfloat16
F32R = mybir.dt.float32r

B, T, C = 4, 2048, 1024
H, D = 16, 64
N_CORES = 8


def build_core_program(R=T, HPC=8, C_=C):
    KC = C_ // 128            # contraction chunks for QKV matmul
    SUBS = HPC // 2           # head pairs
    MC = 2 * SUBS             # 128-col chunks of q|k sections
    CTXC = HPC * D            # ctx channels owned by this core
    OKC = CTXC // 128         # contraction chunks for out-proj
    NCH = R // 128            # tk/tq 128-chunks
    TQ = min(512, R)          # qkv matmul moving width
    NT = R // TQ
    TSUB = TQ // 128          # v t-chunks per n-tile
    BLK = min(1024, R)        # tq block width for attention/out-proj
    NB = R // BLK
    PW = min(512, BLK)        # PV piece width / shared PSUM tile width
    LCH = BLK // 128          # chunks served by the outer attn pool
    EXP = mybir.ActivationFunctionType.Exp

    nc = bacc.Bacc("TRN2", target_bir_lowering=False, debug=False)

    x_t = nc.dram_tensor("x_t", [C_, R], BF16, kind="ExternalInput")
    w_qkv_c = nc.dram_tensor("w_qkv_c", [C_, 3 * CTXC], BF16, kind="ExternalInput")
    w_out_c = nc.dram_tensor("w_out_c", [CTXC, C_], BF16, kind="ExternalInput")
    y_part = nc.dram_tensor("y_part", [R, C_], F32, kind="ExternalOutput")

    with tile.TileContext(nc) as tc:
        with (
            tc.tile_pool(name="qkv", bufs=1) as qkvp,
            tc.tile_pool(name="vsb", bufs=1) as vsbp,
            tc.tile_pool(name="ctxT", bufs=1) as ctxTp,
            tc.tile_pool(name="wout", bufs=1) as woutp,
            tc.tile_pool(name="attnlo", bufs=1) as attnlo,
            tc.tile_pool(name="smallsb", bufs=2) as smallsb,
            tc.tile_pool(name="yev", bufs=2) as yevp,
            tc.tile_pool(name="scoresps", bufs=2, space="PSUM") as sps,
            tc.tile_pool(name="ps512", bufs=2, space="PSUM") as cpsp,
            tc.tile_pool(name="ctxps", bufs=2, space="PSUM") as ctxps,
            tc.tile_pool(name="wp", bufs=1) as wp,
            tc.tile_pool(name="xp", bufs=NT) as xp,
        ):
            qT = qkvp.tile([128, SUBS, R], BF16)
            kT = qkvp.tile([128, SUBS, R], BF16)
            # v_sb[tk, chunk, pair] = [v_A(64) | ones(64) | v_B(64)]
            v_sb = vsbp.tile([128, NCH, SUBS, 192], BF16)
            ctx_T = ctxTp.tile([128, OKC, R], BF16)
            w_out_sb = woutp.tile([128, OKC, C_], BF16)
            nc.gpsimd.memset(v_sb[:, :, :, 64:128], 1.0)

            def ps512():
                return cpsp.tile([128, PW], F32, name="ps512", tag="ps512")

            def ctx_tile():
                return ctxps.tile([128, PW], F32, name="ctx_ps", tag="ctx_ps")

            def emit_outproj(gm):
                for yo in range(0, C_, PW):
                    yp = ps512()
                    for kc in range(OKC):
                        nc.tensor.matmul(
                            yp,
                            lhsT=ctx_T[:, kc, 128 * gm:128 * (gm + 1)],
                            rhs=w_out_sb[:, kc, yo:yo + PW],
                            start=(kc == 0), stop=(kc == OKC - 1),
                        )
                    ye = yevp.tile([128, PW], F32, name="ye", tag="ye")
                    nc.vector.tensor_copy(out=ye, in_=yp)
                    nc.sync.dma_start(
                        out=y_part[128 * gm:128 * (gm + 1), yo:yo + PW],
                        in_=ye,
                    )

            def do_block(jb, pair_bls, attnhi):
                """One tq block.  PE work that does not depend on a fresh
                exp (PV of already-exp'd chunks, plus per-pair backlog
                closures: QKV filler / previous block's out-proj) is pumped
                between score-chunk emissions so the in-order PE queue
                never sits on an ACT wait.  Each pair's backlog is fully
                drained by its end, so a closure another pair depends on
                must be placed at least one pair early."""
                blo, bhi = BLK * jb, BLK * (jb + 1)
                chunks = [i for i in range(NCH) if 128 * i < bhi]
                pieces = list(range(0, BLK, PW))
                last_t = {
                    p: max(i for i in chunks
                           if max(0, 128 * i - blo) < p + PW)
                    for p in pieces
                }
                for sub in range(SUBS):
                    deferred = []
                    pair_bl = list(reversed(pair_bls[sub]))

                    def pump(lag=1):
                        # one exp-independent backlog item absorbs the ACT
                        # latency, then PV down to `lag` pending chunks
                        if pair_bl:
                            pair_bl.pop()()
                        while len(deferred) > lag:
                            deferred.pop(0)()

                    def sc_chunk(i):
                        lo = max(blo, 128 * i)
                        c0 = lo - blo
                        width = bhi - lo
                        wi = min(BLK, R - 128 * i)
                        pool = attnlo if i < LCH else attnhi
                        ps = {}
                        at = {}
                        for hs in (0, 1):
                            at[hs] = pool.tile(
                                [128, wi], BF16,
                                name=f"at{hs}_{i}", tag=f"a{hs}_{i}")
                            ps[hs] = sps.tile([128, BLK], F32,
                                              name="sc_ps", tag="sc_ps")
                        for p in range(0, width, 512):
                            nw = min(512, width - p)
                            for hs in (0, 1):
                                r0 = 64 * hs
                                nc.tensor.matmul(
                                    ps[hs][:, p:p + nw],
                                    lhsT=kT[r0:r0 + 64, sub,
                                            128 * i:128 * (i + 1)],
                                    rhs=qT[r0:r0 + 64, sub,
                                           lo + p:lo + p + nw],
                                    start=True, stop=True,
                                    tile_position=(r0, 0),
                                )
                        for hs in (0, 1):
                            nc.scalar.activation(at[hs][:, 0:width],
                                                 ps[hs][:, 0:width],
                                                 EXP, scale=0.125)
                            if lo == 128 * i:  # diagonal: zero upper tri
                                nc.gpsimd.affine_select(
                                    out=at[hs][:, 0:128],
                                    in_=at[hs][:, 0:128],
                                    compare_op=mybir.AluOpType.is_ge,
                                    fill=0.0, base=0,
                                    pattern=[[1, 128]],
                                    channel_multiplier=-1,
                                )
                        return at

                    def pv(i, at, p, cps):
                        def emit():
                            c0 = max(0, 128 * i - blo)
                            s, e = max(c0, p), p + PW
                            for hs in (0, 1):
                                nc.tensor.matmul(
                                    cps[hs][:, s - p:e - p],
                                    lhsT=v_sb[:, i, sub,
                                              64 * hs:64 * hs + 128],
                                    rhs=at[hs][:, s - c0:e - c0],
                                    start=(i == 0),
                                    stop=(i == last_t[p]),
                                )
                        return emit

                    def normalize(cps, p):
                        for hs in (0, 1):
                            # A: ctx rows 0-63, denom 64-127; B flipped
                            cr, dr = (0, 64) if hs == 0 else (64, 0)
                            r0 = 64 * hs
                            rec = smallsb.tile([128, PW], F32, name="rec",
                                               tag="rec")
                            nc.vector.reciprocal_approx_fast(
                                out=rec, in_=cps[hs])
                            nc.vector.tensor_mul(
                                ctx_T[r0:r0 + 64, sub,
                                      blo + p:blo + p + PW],
                                cps[hs][cr:cr + 64, :],
                                rec[dr:dr + 64, :],
                            )

                    p0_chunks = [i for i in chunks
                                 if max(0, 128 * i - blo) < PW]
                    p1_chunks = [i for i in chunks
                                 if max(0, 128 * i - blo) >= PW]
                    two_p = len(pieces) == 2
                    # phase A: piece-0 scores+PV, two-chunk PV lag
                    ctx0 = {0: ctx_tile(), 1: ctx_tile()}
                    pv1 = []
                    for ci, i in enumerate(p0_chunks):
                        at = sc_chunk(i)
                        if ci > 0:
                            pump(lag=2)
                        deferred.append(pv(i, at, 0, ctx0))
                        if two_p:
                            pv1.append((i, at))
                    while deferred:
                        deferred.pop(0)()
                    normalize(ctx0, 0)
                    # phase B: piece-1 scores + all piece-1 PV
                    if two_p:
                        ctx1 = {0: ctx_tile(), 1: ctx_tile()}
                        for (i, at) in pv1:
                            deferred.append(pv(i, at, PW, ctx1))
                        for j in p1_chunks:
                            at = sc_chunk(j)
                            while len(deferred) > 2:
                                deferred.pop(0)()
                            pump(lag=2)
                            deferred.append(pv(j, at, PW, ctx1))
                        while deferred:
                            deferred.pop(0)()
                        normalize(ctx1, PW)
                    while pair_bl:
                        pair_bl.pop()()

            # ---- phase 1 ----
            if True:
                def dma_x(n):
                    tiles = []
                    for kc in range(KC):
                        x_sb = xp.tile([128, TQ], BF16, name=f"x_sb{kc}",
                                       tag=f"x{kc}")
                        nc.sync.dma_start(
                            out=x_sb,
                            in_=x_t[128 * kc:128 * (kc + 1),
                                    n * TQ:(n + 1) * TQ],
                        )
                        tiles.append(x_sb)
                    return tiles

                w_tiles = []
                x_tiles = {0: dma_x(0)}
                for kc in range(KC):
                    w_sb = wp.tile([128, 3 * CTXC], BF16, name=f"w_sb{kc}",
                                   tag=f"w{kc}")
                    nc.sync.dma_start(
                        out=w_sb, in_=w_qkv_c[128 * kc:128 * (kc + 1), :]
                    )
                    w_tiles.append(w_sb)
                    if kc == 3 and NT > 1:
                        x_tiles[1] = dma_x(1)

                def emit_qk_group(n, mc):
                    ps = ps512()
                    for kc in range(KC):
                        nc.tensor.matmul(
                            ps[:, 0:TQ],
                            lhsT=w_tiles[kc][:, 128 * mc:128 * (mc + 1)],
                            rhs=x_tiles[n][kc],
                            start=(kc == 0), stop=(kc == KC - 1),
                        )
                    sec, sub = mc // SUBS, mc % SUBS
                    dest = (qT, kT)[sec]
                    nc.vector.tensor_copy(
                        out=dest[:, sub, n * TQ:(n + 1) * TQ],
                        in_=ps[:, 0:TQ],
                    )

                def emit_v_group(n, ts):
                    vps = ps512()
                    for kc in range(KC):
                        nc.tensor.matmul(
                            vps[:, 0:CTXC],
                            lhsT=x_tiles[n][kc][:, 128 * ts:128 * (ts + 1)],
                            rhs=w_tiles[kc][:, 2 * CTXC:3 * CTXC],
                            start=(kc == 0), stop=(kc == KC - 1),
                        )
                    i = n * TSUB + ts
                    for s in range(SUBS):
                        nc.vector.tensor_copy(
                            out=v_sb[:, i, s, 0:64],
                            in_=vps[:, 128 * s:128 * s + 64],
                        )
                        nc.vector.tensor_copy(
                            out=v_sb[:, i, s, 128:192],
                            in_=vps[:, 128 * s + 64:128 * s + 128],
                        )

                head_ns = [n for n in range(NT) if n * TQ < BLK]
                fill_ns = [n for n in range(NT) if n * TQ >= BLK]
                # minimal head: pair 0's q,k + block 0's v, then attention
                # starts.  Everything else is backlog, balanced between the
                # ACT-light block 0 and the ACT-bound later blocks.  A
                # closure pair s depends on goes to pair s-1 (or earlier).
                for n in head_ns:
                    emit_qk_group(n, 0)
                    emit_qk_group(n, SUBS)
                for n in head_ns:
                    for ts in range(TSUB):
                        emit_v_group(n, ts)
                for n in fill_ns:
                    x_tiles[n] = dma_x(n)
                for kc in range(OKC):  # not needed until the first out-proj
                    nc.sync.dma_start(
                        out=w_out_sb[:, kc, :],
                        in_=w_out_c[128 * kc:128 * (kc + 1), :],
                    )

                def qk_pair(ns, sub):
                    out = []
                    for n in ns:
                        out.append(partial(emit_qk_group, n, sub))
                        out.append(partial(emit_qk_group, n, SUBS + sub))
                    return out

                def v_tiles(ns):
                    return [partial(emit_v_group, n, ts)
                            for n in ns for ts in range(TSUB)]

                # dependency-free filler, spread round-robin
                free0 = qk_pair(fill_ns, 0) + v_tiles(fill_ns[:-1])
                if NB == 1:
                    free0 += v_tiles(fill_ns[-1:])
                pb0 = [qk_pair(head_ns, s + 1) if s + 1 < SUBS else []
                       for s in range(SUBS)]
                for idx, it in enumerate(free0):
                    pb0[idx % SUBS].append(it)
                do_block(0, pb0, attnlo)

                prev_gms = [m for m in range(LCH)]
                for jb in range(1, NB):
                    # pair s's q,k land one pair early so their DVE casts
                    # are done before pair s reads qT/kT
                    pb = [[] for _ in range(SUBS)]
                    if jb == 1:
                        for s in range(1, SUBS):
                            pb[s - 1] += qk_pair(fill_ns, s)
                        pb[0] += v_tiles(fill_ns[-1:])
                    ops = [partial(emit_outproj, g) for g in prev_gms]
                    start = 1 if SUBS > 1 else 0
                    for idx, it in enumerate(ops):
                        pb[start + idx % (SUBS - start)].append(it)
                    do_block(jb, pb, attnlo)
                    prev_gms = [LCH * jb + m for m in range(LCH)]
                for gm in prev_gms:
                    emit_outproj(gm)

    nc.finalize()
    return nc


def make_in_maps(x, w_qkv, w_out):
    x = np.asarray(x, dtype=np.float32)
    w_qkv = np.asarray(w_qkv, dtype=np.float32)
    w_out = np.asarray(w_out, dtype=np.float32)
    in_maps = []
    for core in range(N_CORES):
        b, g = core // 2, core % 2
        cols = slice(512 * g, 512 * (g + 1))
        wq = np.ascontiguousarray(
            np.concatenate(
                [w_qkv[:, cols], w_qkv[:, 1024:][:, cols], w_qkv[:, 2048:][:, cols]],
                axis=1,
            )
        )
        in_maps.append({
            "x_t": np.ascontiguousarray(x[b].T).astype(ml_dtypes.bfloat16),
            "w_qkv_c": wq.astype(ml_dtypes.bfloat16),
            "w_out_c": np.ascontiguousarray(
                w_out[512 * g:512 * (g + 1), :]).astype(ml_dtypes.bfloat16),
        })
    return in_maps


_NC_CACHE = None
LAST_RESULT = None


def kernel(x, w_qkv, w_out):
    global _NC_CACHE, LAST_RESULT
    if _NC_CACHE is None:
        _NC_CACHE = build_core_program()
    nc = _NC_CACHE
    in_maps = make_in_maps(x, w_qkv, w_out)
    res = run_bass_kernel_spmd(nc, in_maps, list(range(N_CORES)))
    LAST_RESULT = res
    outs = [r["y_part"] for r in res.results]
    y = np.stack([outs[2 * b] + outs[2 * b + 1] for b in range(B)], axis=0)
    return y.astype(np.float32)


# revision 34
# speedup vs baseline: 1.2162x; 1.2162x over previous
"""Causal self-attention (B=4, T=2048, C=1024, H=16, D=64) on 8 TRN2 NeuronCores.

Sharding: core i handles batch b = i//2 and head-group g = i%2 (8 of the 16
heads).  Each core computes the QKV projection for its batch restricted to its
heads' columns, runs causal attention for its 8 heads, and produces a partial
output projection y_part = ctx_g @ w_out[rows of g].  The two partials per
batch are summed on the host (y[b] = y_part[2b] + y_part[2b+1]).

The kernel is PE-cycle-bound (the chip power-throttles the PE clock to ~50%
duty under sustained 8-core matmul load), so the layout minimizes PE work,
and the attention inner loop is ACT(exp)-latency-coupled, so exp-independent
matmuls are interleaved as backlog to keep the in-order PE queue fed:
  - x and w_qkv are cast to bf16 on the host (halves the DMA-bound head;
    total rel err ~0.005 vs the 0.02 budget).
  - q,k are produced transposed ([d, t]); v is produced directly in
    [t, ch] layout (stationary = x chunks), no PE transposes.  Per
    (chunk, head-pair) v is stored as [v_A | ones | v_B]; head A's PV
    stationary [v_A|ones] yields ctx in PSUM rows 0-63 and the softmax
    denominator in rows 64-127, head B's [ones|v_B] the reverse.
  - QK^T for a pair of heads is issued as row-tiled matmuls (head A in
    array rows 0-63 / tile_position (0,0), head B in rows 64-127 /
    (64,0), separate PSUM banks).
  - causal mask applied AFTER exp by zeroing the upper triangle of the
    diagonal 128-block with gpsimd affine_select (raw scores are |s|<~10
    so exp before masking is safe).
  - PV accumulates exact causal ranges (no zero-padding matmuls), one
    512-wide piece at a time, deferred two chunks behind the score
    matmuls so it never waits on a fresh exp.
  - only head-pair 0's q,k projections run before attention starts; all
    other QKV work + the previous block's output projection are pumped
    as per-pair backlog between score chunks.  A backlog closure that a
    later PE instruction depends on is placed at least one pair early
    (the in-order PE queue deadlocks otherwise), and the x pool holds
    all NT n-tiles so no x-DMA WAR can cycle through the backlog.
"""

from functools import partial

import numpy as np
import ml_dtypes

import concourse.bass as bass
import concourse.mybir as mybir
from concourse import bacc, tile
from concourse.bass_utils import run_bass_kernel_spmd

F32 = mybir.dt.float32
BF16 = mybir.dt.bfloat16
F32R = mybir.dt.float32r

B, T, C = 4, 2048, 1024
H, D = 16, 64
N_CORES = 8


def build_core_program(R=T, HPC=8, C_=C):
    KC = C_ // 128            # contraction chunks for QKV matmul
    SUBS = HPC // 2           # head pairs
    MC = 2 * SUBS             # 128-col chunks of q|k sections
    CTXC = HPC * D            # ctx channels owned by this core
    OKC = CTXC // 128         # contraction chunks for out-proj
    NCH = R // 128            # tk/tq 128-chunks
    TQ = min(512, R)          # qkv matmul moving width
    NT = R // TQ
    TSUB = TQ // 128          # v t-chunks per n-tile
    BLK = min(1024, R)        # tq block width for attention/out-proj
    NB = R // BLK
    PW = min(512, BLK)        # PV piece width / shared PSUM tile width
    LCH = BLK // 128          # chunks served by the outer attn pool
    EXP = mybir.ActivationFunctionType.Exp

    nc = bacc.Bacc("TRN2", target_bir_lowering=False, debug=False)

    x_t = nc.dram_tensor("x_t", [C_, R], BF16, kind="ExternalInput")
    w_qkv_c = nc.dram_tensor("w_qkv_c", [C_, 3 * CTXC], BF16, kind="ExternalInput")
    w_out_c = nc.dram_tensor("w_out_c", [CTXC, C_], BF16, kind="ExternalInput")
    y_part = nc.dram_tensor("y_part", [R, C_], F32, kind="ExternalOutput")

    with tile.TileContext(nc) as tc:
        with (
            tc.tile_pool(name="qkv", bufs=1) as qkvp,
            tc.tile_pool(name="vsb", bufs=1) as vsbp,
            tc.tile_pool(name="ctxT", bufs=1) as ctxTp,
            tc.tile_pool(name="wout", bufs=1) as woutp,
            tc.tile_pool(name="attnlo", bufs=1) as attnlo,
            tc.tile_pool(name="attn2", bufs=2) as attn2,
            tc.tile_pool(name="smallsb", bufs=2) as smallsb,
            tc.tile_pool(name="yev", bufs=2) as yevp,
            tc.tile_pool(name="scoresps", bufs=2, space="PSUM") as sps,
            tc.tile_pool(name="ps512", bufs=2, space="PSUM") as cpsp,
            tc.tile_pool(name="ctxps", bufs=2, space="PSUM") as ctxps,
            tc.tile_pool(name="wp", bufs=1) as wp,
            tc.tile_pool(name="xp", bufs=NT) as xp,
        ):
            qT = qkvp.tile([128, SUBS, R], BF16)
            kT = qkvp.tile([128, SUBS, R], BF16)
            # v_sb[tk, chunk, pair] = [v_A(64) | ones(64) | v_B(64)]
            v_sb = vsbp.tile([128, NCH, SUBS, 192], BF16)
            ctx_T = ctxTp.tile([128, OKC, R], BF16)
            w_out_sb = woutp.tile([128, OKC, C_], BF16)
            nc.gpsimd.memset(v_sb[:, :, :, 64:128], 1.0)

            def ps512():
                return cpsp.tile([128, PW], F32, name="ps512", tag="ps512")

            def ctx_tile():
                return ctxps.tile([128, PW], F32, name="ctx_ps", tag="ctx_ps")

            def emit_outproj(gm):
                for yo in range(0, C_, PW):
                    yp = ps512()
                    for kc in range(OKC):
                        nc.tensor.matmul(
                            yp,
                            lhsT=ctx_T[:, kc, 128 * gm:128 * (gm + 1)],
                            rhs=w_out_sb[:, kc, yo:yo + PW],
                            start=(kc == 0), stop=(kc == OKC - 1),
                        )
                    ye = yevp.tile([128, PW], F32, name="ye", tag="ye")
                    nc.vector.tensor_copy(out=ye, in_=yp)
                    nc.sync.dma_start(
                        out=y_part[128 * gm:128 * (gm + 1), yo:yo + PW],
                        in_=ye,
                    )

            def do_block(jb, pair_bls, attnhi):
                """One tq block.  PE work that does not depend on a fresh
                exp (PV of already-exp'd chunks, plus per-pair backlog
                closures: QKV filler / previous block's out-proj) is pumped
                between score-chunk emissions so the in-order PE queue
                never sits on an ACT wait.  Each pair's backlog is fully
                drained by its end, so a closure another pair depends on
                must be placed at least one pair early."""
                blo, bhi = BLK * jb, BLK * (jb + 1)
                chunks = [i for i in range(NCH) if 128 * i < bhi]
                pieces = list(range(0, BLK, PW))
                last_t = {
                    p: max(i for i in chunks
                           if max(0, 128 * i - blo) < p + PW)
                    for p in pieces
                }
                for sub in range(SUBS):
                    deferred = []
                    pair_bl = list(reversed(pair_bls[sub]))

                    def pump(lag=1):
                        # one exp-independent backlog item absorbs the ACT
                        # latency, then PV down to `lag` pending chunks
                        if pair_bl:
                            pair_bl.pop()()
                        while len(deferred) > lag:
                            deferred.pop(0)()

                    def sc_chunk(i):
                        lo = max(blo, 128 * i)
                        c0 = lo - blo
                        width = bhi - lo
                        wi = min(BLK, R - 128 * i)
                        # first 4 chunk tags double-buffered: the next
                        # pair's exp does not wait this pair's PV read
                        pool = (attn2 if i < 3 else
                                attnlo if i < LCH else attnhi)
                        ps = {}
                        at = {}
                        for hs in (0, 1):
                            at[hs] = pool.tile(
                                [128, wi], BF16,
                                name=f"at{hs}_{i}", tag=f"a{hs}_{i}")
                            ps[hs] = sps.tile([128, BLK], F32,
                                              name="sc_ps", tag="sc_ps")
                        for p in range(0, width, 512):
                            nw = min(512, width - p)
                            for hs in (0, 1):
                                r0 = 64 * hs
                                nc.tensor.matmul(
                                    ps[hs][:, p:p + nw],
                                    lhsT=kT[r0:r0 + 64, sub,
                                            128 * i:128 * (i + 1)],
                                    rhs=qT[r0:r0 + 64, sub,
                                           lo + p:lo + p + nw],
                                    start=True, stop=True,
                                    tile_position=(r0, 0),
                                )
                        for hs in (0, 1):
                            nc.scalar.activation(at[hs][:, 0:width],
                                                 ps[hs][:, 0:width],
                                                 EXP, scale=0.125)
                            if lo == 128 * i:  # diagonal: zero upper tri
                                nc.gpsimd.affine_select(
                                    out=at[hs][:, 0:128],
                                    in_=at[hs][:, 0:128],
                                    compare_op=mybir.AluOpType.is_ge,
                                    fill=0.0, base=0,
                                    pattern=[[1, 128]],
                                    channel_multiplier=-1,
                                )
                        return at

                    def pv(i, at, p, cps):
                        def emit():
                            c0 = max(0, 128 * i - blo)
                            s, e = max(c0, p), p + PW
                            for hs in (0, 1):
                                nc.tensor.matmul(
                                    cps[hs][:, s - p:e - p],
                                    lhsT=v_sb[:, i, sub,
                                              64 * hs:64 * hs + 128],
                                    rhs=at[hs][:, s - c0:e - c0],
                                    start=(i == 0),
                                    stop=(i == last_t[p]),
                                )
                        return emit

                    def normalize(cps, p):
                        for hs in (0, 1):
                            # A: ctx rows 0-63, denom 64-127; B flipped
                            cr, dr = (0, 64) if hs == 0 else (64, 0)
                            r0 = 64 * hs
                            rec = smallsb.tile([128, PW], F32, name="rec",
                                               tag="rec")
                            nc.vector.reciprocal_approx_fast(
                                out=rec, in_=cps[hs])
                            nc.vector.tensor_mul(
                                ctx_T[r0:r0 + 64, sub,
                                      blo + p:blo + p + PW],
                                cps[hs][cr:cr + 64, :],
                                rec[dr:dr + 64, :],
                            )

                    p0_chunks = [i for i in chunks
                                 if max(0, 128 * i - blo) < PW]
                    p1_chunks = [i for i in chunks
                                 if max(0, 128 * i - blo) >= PW]
                    two_p = len(pieces) == 2
                    # phase A: piece-0 scores+PV, two-chunk PV lag
                    ctx0 = {0: ctx_tile(), 1: ctx_tile()}
                    pv1 = []
                    for ci, i in enumerate(p0_chunks):
                        at = sc_chunk(i)
                        if ci > 0:
                            pump(lag=2)
                        deferred.append(pv(i, at, 0, ctx0))
                        if two_p:
                            pv1.append((i, at))
                    while deferred:
                        deferred.pop(0)()
                    normalize(ctx0, 0)
                    # phase B: piece-1 scores + all piece-1 PV
                    if two_p:
                        ctx1 = {0: ctx_tile(), 1: ctx_tile()}
                        for (i, at) in pv1:
                            deferred.append(pv(i, at, PW, ctx1))
                        for j in p1_chunks:
                            at = sc_chunk(j)
                            while len(deferred) > 2:
                                deferred.pop(0)()
                            pump(lag=2)
                            deferred.append(pv(j, at, PW, ctx1))
                        while deferred:
                            deferred.pop(0)()
                        normalize(ctx1, PW)
                    while pair_bl:
                        pair_bl.pop()()

            # ---- phase 1 ----
            if True:
                def dma_x(n):
                    tiles = []
                    for kc in range(KC):
                        x_sb = xp.tile([128, TQ], BF16, name=f"x_sb{kc}",
                                       tag=f"x{kc}")
                        nc.sync.dma_start(
                            out=x_sb,
                            in_=x_t[128 * kc:128 * (kc + 1),
                                    n * TQ:(n + 1) * TQ],
                        )
                        tiles.append(x_sb)
                    return tiles

                w_tiles = []
                x_tiles = {0: dma_x(0)}
                for kc in range(KC):
                    w_sb = wp.tile([128, 3 * CTXC], BF16, name=f"w_sb{kc}",
                                   tag=f"w{kc}")
                    w_tiles.append(w_sb)
                # two-phase w DMA: the columns pair 0's first score chunk
                # needs (whole q section + k sub 0) land first, so
                # attention starts ~7us earlier
                QH = CTXC + 128
                for kc in range(KC):
                    nc.sync.dma_start(
                        out=w_tiles[kc][:, 0:QH],
                        in_=w_qkv_c[128 * kc:128 * (kc + 1), 0:QH],
                    )
                if NT > 1:
                    x_tiles[1] = dma_x(1)
                for kc in range(KC):
                    nc.sync.dma_start(
                        out=w_tiles[kc][:, QH:3 * CTXC],
                        in_=w_qkv_c[128 * kc:128 * (kc + 1), QH:3 * CTXC],
                    )

                def emit_qk_group(n, mc):
                    ps = ps512()
                    for kc in range(KC):
                        nc.tensor.matmul(
                            ps[:, 0:TQ],
                            lhsT=w_tiles[kc][:, 128 * mc:128 * (mc + 1)],
                            rhs=x_tiles[n][kc],
                            start=(kc == 0), stop=(kc == KC - 1),
                        )
                    sec, sub = mc // SUBS, mc % SUBS
                    dest = (qT, kT)[sec]
                    nc.vector.tensor_copy(
                        out=dest[:, sub, n * TQ:(n + 1) * TQ],
                        in_=ps[:, 0:TQ],
                    )

                def emit_v_group(n, ts):
                    vps = ps512()
                    for kc in range(KC):
                        nc.tensor.matmul(
                            vps[:, 0:CTXC],
                            lhsT=x_tiles[n][kc][:, 128 * ts:128 * (ts + 1)],
                            rhs=w_tiles[kc][:, 2 * CTXC:3 * CTXC],
                            start=(kc == 0), stop=(kc == KC - 1),
                        )
                    i = n * TSUB + ts
                    for s in range(SUBS):
                        nc.vector.tensor_copy(
                            out=v_sb[:, i, s, 0:64],
                            in_=vps[:, 128 * s:128 * s + 64],
                        )
                        nc.vector.tensor_copy(
                            out=v_sb[:, i, s, 128:192],
                            in_=vps[:, 128 * s + 64:128 * s + 128],
                        )

                head_ns = [n for n in range(NT) if n * TQ < BLK]
                fill_ns = [n for n in range(NT) if n * TQ >= BLK]
                # minimal head: pair 0's q,k + block 0's v, then attention
                # starts.  Everything else is backlog, balanced between the
                # ACT-light block 0 and the ACT-bound later blocks.  A
                # closure pair s depends on goes to pair s-1 (or earlier).
                for n in head_ns:
                    emit_qk_group(n, 0)
                    emit_qk_group(n, SUBS)
                for n in head_ns:
                    for ts in range(TSUB):
                        emit_v_group(n, ts)
                for n in fill_ns:
                    x_tiles[n] = dma_x(n)
                for kc in range(OKC):  # not needed until the first out-proj
                    nc.sync.dma_start(
                        out=w_out_sb[:, kc, :],
                        in_=w_out_c[128 * kc:128 * (kc + 1), :],
                    )

                def qk_pair(ns, sub):
                    out = []
                    for n in ns:
                        out.append(partial(emit_qk_group, n, sub))
                        out.append(partial(emit_qk_group, n, SUBS + sub))
                    return out

                def v_tiles(ns):
                    return [partial(emit_v_group, n, ts)
                            for n in ns for ts in range(TSUB)]

                # dependency-free filler, spread round-robin
                free0 = qk_pair(fill_ns, 0) + v_tiles(fill_ns[:-1])
                if NB == 1:
                    free0 += v_tiles(fill_ns[-1:])
                pb0 = [qk_pair(head_ns, s + 1) if s + 1 < SUBS else []
                       for s in range(SUBS)]
                for idx, it in enumerate(free0):
                    pb0[idx % SUBS].append(it)
                do_block(0, pb0, attnlo)

                prev_gms = [m for m in range(LCH)]
                for jb in range(1, NB):
                    # pair s's q,k land one pair early so their DVE casts
                    # are done before pair s reads qT/kT
                    pb = [[] for _ in range(SUBS)]
                    if jb == 1:
                        for s in range(1, SUBS):
                            pb[s - 1] += qk_pair(fill_ns, s)
                        pb[0] += v_tiles(fill_ns[-1:])
                    ops = [partial(emit_outproj, g) for g in prev_gms]
                    start = 1 if SUBS > 1 else 0
                    for idx, it in enumerate(ops):
                        pb[start + idx % (SUBS - start)].append(it)
                    do_block(jb, pb, attnlo)
                    prev_gms = [LCH * jb + m for m in range(LCH)]
                for gm in prev_gms:
                    emit_outproj(gm)

    nc.finalize()
    return nc


def make_in_maps(x, w_qkv, w_out):
    x = np.asarray(x, dtype=np.float32)
    w_qkv = np.asarray(w_qkv, dtype=np.float32)
    w_out = np.asarray(w_out, dtype=np.float32)
    in_maps = []
    for core in range(N_CORES):
        b, g = core // 2, core % 2
        cols = slice(512 * g, 512 * (g + 1))
        wq = np.ascontiguousarray(
            np.concatenate(
                [w_qkv[:, cols], w_qkv[:, 1024:][:, cols], w_qkv[:, 2048:][:, cols]],
                axis=1,
            )
        )
        in_maps.append({
            "x_t": np.ascontiguousarray(x[b].T).astype(ml_dtypes.bfloat16),
            "w_qkv_c": wq.astype(ml_dtypes.bfloat16),
            "w_out_c": np.ascontiguousarray(
                w_out[512 * g:512 * (g + 1), :]).astype(ml_dtypes.bfloat16),
        })
    return in_maps


_NC_CACHE = None
LAST_RESULT = None


def kernel(x, w_qkv, w_out):
    global _NC_CACHE, LAST_RESULT
    if _NC_CACHE is None:
        _NC_CACHE = build_core_program()
    nc = _NC_CACHE
    in_maps = make_in_maps(x, w_qkv, w_out)
    res = run_bass_kernel_spmd(nc, in_maps, list(range(N_CORES)))
    LAST_RESULT = res
    outs = [r["y_part"] for r in res.results]
    y = np.stack([outs[2 * b] + outs[2 * b + 1] for b in range(B)], axis=0)
    return y.astype(np.float32)


# revision 35
# speedup vs baseline: 1.2332x; 1.0140x over previous
"""Causal self-attention (B=4, T=2048, C=1024, H=16, D=64) on 8 TRN2 NeuronCores.

Sharding: core i handles batch b = i//2 and head-group g = i%2 (8 of the 16
heads).  Each core computes the QKV projection for its batch restricted to its
heads' columns, runs causal attention for its 8 heads, and produces a partial
output projection y_part = ctx_g @ w_out[rows of g].  The two partials per
batch are summed on the host (y[b] = y_part[2b] + y_part[2b+1]).

The kernel is PE-cycle-bound (the chip power-throttles the PE clock to ~50%
duty under sustained 8-core matmul load), so the layout minimizes PE work,
and the attention inner loop is ACT(exp)-latency-coupled, so exp-independent
matmuls are interleaved as backlog to keep the in-order PE queue fed:
  - x and w_qkv are cast to bf16 on the host (halves the DMA-bound head;
    total rel err ~0.005 vs the 0.02 budget).
  - q,k are produced transposed ([d, t]); v is produced directly in
    [t, ch] layout (stationary = x chunks), no PE transposes.  Per
    (chunk, head-pair) v is stored as [v_A | ones | v_B]; head A's PV
    stationary [v_A|ones] yields ctx in PSUM rows 0-63 and the softmax
    denominator in rows 64-127, head B's [ones|v_B] the reverse.
  - QK^T for a pair of heads is issued as row-tiled matmuls (head A in
    array rows 0-63 / tile_position (0,0), head B in rows 64-127 /
    (64,0), separate PSUM banks).
  - causal mask applied AFTER exp by zeroing the upper triangle of the
    diagonal 128-block with gpsimd affine_select (raw scores are |s|<~10
    so exp before masking is safe).
  - PV accumulates exact causal ranges (no zero-padding matmuls), one
    512-wide piece at a time, deferred two chunks behind the score
    matmuls so it never waits on a fresh exp.
  - only head-pair 0's q,k projections run before attention starts; all
    other QKV work + the previous block's output projection are pumped
    as per-pair backlog between score chunks.  A backlog closure that a
    later PE instruction depends on is placed at least one pair early
    (the in-order PE queue deadlocks otherwise), and the x pool holds
    all NT n-tiles so no x-DMA WAR can cycle through the backlog.
"""

from functools import partial

import numpy as np
import ml_dtypes

import concourse.bass as bass
import concourse.mybir as mybir
from concourse import bacc, tile
from concourse.bass_utils import run_bass_kernel_spmd

F32 = mybir.dt.float32
BF16 = mybir.dt.bfloat16
F32R = mybir.dt.float32r

B, T, C = 4, 2048, 1024
H, D = 16, 64
N_CORES = 8


def build_core_program(R=T, HPC=8, C_=C):
    KC = C_ // 128            # contraction chunks for QKV matmul
    SUBS = HPC // 2           # head pairs
    MC = 2 * SUBS             # 128-col chunks of q|k sections
    CTXC = HPC * D            # ctx channels owned by this core
    OKC = CTXC // 128         # contraction chunks for out-proj
    NCH = R // 128            # tk/tq 128-chunks
    TQ = min(512, R)          # qkv matmul moving width
    NT = R // TQ
    TSUB = TQ // 128          # v t-chunks per n-tile
    BLK = min(1024, R)        # tq block width for attention/out-proj
    NB = R // BLK
    PW = min(512, BLK)        # PV piece width / shared PSUM tile width
    LCH = BLK // 128          # chunks served by the outer attn pool
    EXP = mybir.ActivationFunctionType.Exp

    nc = bacc.Bacc("TRN2", target_bir_lowering=False, debug=False)

    x_t = nc.dram_tensor("x_t", [C_, R], BF16, kind="ExternalInput")
    w_qkv_c = nc.dram_tensor("w_qkv_c", [C_, 3 * CTXC], BF16, kind="ExternalInput")
    w_out_c = nc.dram_tensor("w_out_c", [CTXC, C_], BF16, kind="ExternalInput")
    y_part = nc.dram_tensor("y_part", [R, C_], F32, kind="ExternalOutput")

    with tile.TileContext(nc) as tc:
        with (
            tc.tile_pool(name="qkv", bufs=1) as qkvp,
            tc.tile_pool(name="vsb", bufs=1) as vsbp,
            tc.tile_pool(name="ctxT", bufs=1) as ctxTp,
            tc.tile_pool(name="wout", bufs=1) as woutp,
            tc.tile_pool(name="attnlo", bufs=1) as attnlo,
            tc.tile_pool(name="attn2", bufs=2) as attn2,
            tc.tile_pool(name="smallsb", bufs=2) as smallsb,
            tc.tile_pool(name="yev", bufs=2) as yevp,
            tc.tile_pool(name="scoresps", bufs=2, space="PSUM") as sps,
            tc.tile_pool(name="ps512", bufs=2, space="PSUM") as cpsp,
            tc.tile_pool(name="ctxps", bufs=2, space="PSUM") as ctxps,
            tc.tile_pool(name="wp", bufs=1) as wp,
            tc.tile_pool(name="xp", bufs=NT) as xp,
        ):
            qT = qkvp.tile([128, SUBS, R], BF16)
            kT = qkvp.tile([128, SUBS, R], BF16)
            # v_sb[tk, chunk, pair] = [v_A(64) | ones(64) | v_B(64)]
            v_sb = vsbp.tile([128, NCH, SUBS, 192], BF16)
            ctx_T = ctxTp.tile([128, OKC, R], BF16)
            w_out_sb = woutp.tile([128, OKC, C_], BF16)
            nc.gpsimd.memset(v_sb[:, :, :, 64:128], 1.0)

            def ps512():
                return cpsp.tile([128, PW], F32, name="ps512", tag="ps512")

            def ctx_tile():
                return ctxps.tile([128, PW], F32, name="ctx_ps", tag="ctx_ps")

            def emit_outproj(gm):
                for yo in range(0, C_, PW):
                    yp = ps512()
                    for kc in range(OKC):
                        nc.tensor.matmul(
                            yp,
                            lhsT=ctx_T[:, kc, 128 * gm:128 * (gm + 1)],
                            rhs=w_out_sb[:, kc, yo:yo + PW],
                            start=(kc == 0), stop=(kc == OKC - 1),
                        )
                    ye = yevp.tile([128, PW], F32, name="ye", tag="ye")
                    nc.vector.tensor_copy(out=ye, in_=yp)
                    nc.sync.dma_start(
                        out=y_part[128 * gm:128 * (gm + 1), yo:yo + PW],
                        in_=ye,
                    )

            def do_block(jb, pair_bls, attnhi, self_ops=()):
                """One tq block.  PE work that does not depend on a fresh
                exp (PV of already-exp'd chunks, plus per-pair backlog
                closures: QKV filler / previous block's out-proj) is pumped
                between score-chunk emissions so the in-order PE queue
                never sits on an ACT wait.  Each pair's backlog is fully
                drained by its end, so a closure another pair depends on
                must be placed at least one pair early."""
                blo, bhi = BLK * jb, BLK * (jb + 1)
                chunks = [i for i in range(NCH) if 128 * i < bhi]
                pieces = list(range(0, BLK, PW))
                last_t = {
                    p: max(i for i in chunks
                           if max(0, 128 * i - blo) < p + PW)
                    for p in pieces
                }
                for sub in range(SUBS):
                    deferred = []
                    pair_bl = list(reversed(pair_bls[sub]))

                    def pump(lag=1):
                        # one exp-independent backlog item absorbs the ACT
                        # latency, then PV down to `lag` pending chunks
                        if pair_bl:
                            pair_bl.pop()()
                        while len(deferred) > lag:
                            deferred.pop(0)()

                    def sc_chunk(i):
                        lo = max(blo, 128 * i)
                        c0 = lo - blo
                        width = bhi - lo
                        wi = min(BLK, R - 128 * i)
                        # first 4 chunk tags double-buffered: the next
                        # pair's exp does not wait this pair's PV read
                        pool = (attn2 if i < 3 else
                                attnlo if i < LCH else attnhi)
                        ps = {}
                        at = {}
                        for hs in (0, 1):
                            at[hs] = pool.tile(
                                [128, wi], BF16,
                                name=f"at{hs}_{i}", tag=f"a{hs}_{i}")
                            ps[hs] = sps.tile([128, BLK], F32,
                                              name="sc_ps", tag="sc_ps")
                        for p in range(0, width, 512):
                            nw = min(512, width - p)
                            for hs in (0, 1):
                                r0 = 64 * hs
                                nc.tensor.matmul(
                                    ps[hs][:, p:p + nw],
                                    lhsT=kT[r0:r0 + 64, sub,
                                            128 * i:128 * (i + 1)],
                                    rhs=qT[r0:r0 + 64, sub,
                                           lo + p:lo + p + nw],
                                    start=True, stop=True,
                                    tile_position=(r0, 0),
                                )
                        for hs in (0, 1):
                            nc.scalar.activation(at[hs][:, 0:width],
                                                 ps[hs][:, 0:width],
                                                 EXP, scale=0.125)
                            if lo == 128 * i:  # diagonal: zero upper tri
                                nc.gpsimd.affine_select(
                                    out=at[hs][:, 0:128],
                                    in_=at[hs][:, 0:128],
                                    compare_op=mybir.AluOpType.is_ge,
                                    fill=0.0, base=0,
                                    pattern=[[1, 128]],
                                    channel_multiplier=-1,
                                )
                        return at

                    def pv(i, at, p, cps):
                        def emit():
                            c0 = max(0, 128 * i - blo)
                            s, e = max(c0, p), p + PW
                            for hs in (0, 1):
                                nc.tensor.matmul(
                                    cps[hs][:, s - p:e - p],
                                    lhsT=v_sb[:, i, sub,
                                              64 * hs:64 * hs + 128],
                                    rhs=at[hs][:, s - c0:e - c0],
                                    start=(i == 0),
                                    stop=(i == last_t[p]),
                                )
                        return emit

                    def normalize(cps, p):
                        for hs in (0, 1):
                            # A: ctx rows 0-63, denom 64-127; B flipped
                            cr, dr = (0, 64) if hs == 0 else (64, 0)
                            r0 = 64 * hs
                            rec = smallsb.tile([128, PW], F32, name="rec",
                                               tag="rec")
                            nc.vector.reciprocal_approx_fast(
                                out=rec, in_=cps[hs])
                            nc.vector.tensor_mul(
                                ctx_T[r0:r0 + 64, sub,
                                      blo + p:blo + p + PW],
                                cps[hs][cr:cr + 64, :],
                                rec[dr:dr + 64, :],
                            )

                    p0_chunks = [i for i in chunks
                                 if max(0, 128 * i - blo) < PW]
                    p1_chunks = [i for i in chunks
                                 if max(0, 128 * i - blo) >= PW]
                    two_p = len(pieces) == 2
                    # phase A: piece-0 scores+PV, two-chunk PV lag
                    ctx0 = {0: ctx_tile(), 1: ctx_tile()}
                    pv1 = []
                    for ci, i in enumerate(p0_chunks):
                        at = sc_chunk(i)
                        if ci > 0:
                            pump(lag=2)
                        deferred.append(pv(i, at, 0, ctx0))
                        if two_p:
                            pv1.append((i, at))
                    while deferred:
                        deferred.pop(0)()
                    normalize(ctx0, 0)
                    if sub == SUBS - 1:
                        # this block's piece-0 output columns are now fully
                        # normalized; their out-proj overlaps phase B
                        pair_bl.extend(reversed(list(self_ops)))
                    # phase B: piece-1 scores + all piece-1 PV
                    if two_p:
                        ctx1 = {0: ctx_tile(), 1: ctx_tile()}
                        for (i, at) in pv1:
                            deferred.append(pv(i, at, PW, ctx1))
                        for j in p1_chunks:
                            at = sc_chunk(j)
                            while len(deferred) > 2:
                                deferred.pop(0)()
                            pump(lag=2)
                            deferred.append(pv(j, at, PW, ctx1))
                        while deferred:
                            deferred.pop(0)()
                        normalize(ctx1, PW)
                    while pair_bl:
                        pair_bl.pop()()

            # ---- phase 1 ----
            if True:
                def dma_x(n):
                    tiles = []
                    for kc in range(KC):
                        x_sb = xp.tile([128, TQ], BF16, name=f"x_sb{kc}",
                                       tag=f"x{kc}")
                        nc.sync.dma_start(
                            out=x_sb,
                            in_=x_t[128 * kc:128 * (kc + 1),
                                    n * TQ:(n + 1) * TQ],
                        )
                        tiles.append(x_sb)
                    return tiles

                w_tiles = []
                x_tiles = {0: dma_x(0)}
                for kc in range(KC):
                    w_sb = wp.tile([128, 3 * CTXC], BF16, name=f"w_sb{kc}",
                                   tag=f"w{kc}")
                    w_tiles.append(w_sb)
                # two-phase w DMA: the columns pair 0's first score chunk
                # needs (whole q section + k sub 0) land first, so
                # attention starts ~7us earlier
                QH = CTXC + 128
                for kc in range(KC):
                    nc.sync.dma_start(
                        out=w_tiles[kc][:, 0:QH],
                        in_=w_qkv_c[128 * kc:128 * (kc + 1), 0:QH],
                    )
                if NT > 1:
                    x_tiles[1] = dma_x(1)
                for kc in range(KC):
                    nc.sync.dma_start(
                        out=w_tiles[kc][:, QH:3 * CTXC],
                        in_=w_qkv_c[128 * kc:128 * (kc + 1), QH:3 * CTXC],
                    )

                def emit_qk_group(n, mc):
                    ps = ps512()
                    for kc in range(KC):
                        nc.tensor.matmul(
                            ps[:, 0:TQ],
                            lhsT=w_tiles[kc][:, 128 * mc:128 * (mc + 1)],
                            rhs=x_tiles[n][kc],
                            start=(kc == 0), stop=(kc == KC - 1),
                        )
                    sec, sub = mc // SUBS, mc % SUBS
                    dest = (qT, kT)[sec]
                    nc.vector.tensor_copy(
                        out=dest[:, sub, n * TQ:(n + 1) * TQ],
                        in_=ps[:, 0:TQ],
                    )

                def emit_v_group(n, ts):
                    vps = ps512()
                    for kc in range(KC):
                        nc.tensor.matmul(
                            vps[:, 0:CTXC],
                            lhsT=x_tiles[n][kc][:, 128 * ts:128 * (ts + 1)],
                            rhs=w_tiles[kc][:, 2 * CTXC:3 * CTXC],
                            start=(kc == 0), stop=(kc == KC - 1),
                        )
                    i = n * TSUB + ts
                    for s in range(SUBS):
                        nc.vector.tensor_copy(
                            out=v_sb[:, i, s, 0:64],
                            in_=vps[:, 128 * s:128 * s + 64],
                        )
                        nc.vector.tensor_copy(
                            out=v_sb[:, i, s, 128:192],
                            in_=vps[:, 128 * s + 64:128 * s + 128],
                        )

                head_ns = [n for n in range(NT) if n * TQ < BLK]
                fill_ns = [n for n in range(NT) if n * TQ >= BLK]
                # minimal head: pair 0's q,k + block 0's v, then attention
                # starts.  Everything else is backlog, balanced between the
                # ACT-light block 0 and the ACT-bound later blocks.  A
                # closure pair s depends on goes to pair s-1 (or earlier).
                for n in head_ns:
                    emit_qk_group(n, 0)
                    emit_qk_group(n, SUBS)
                for n in head_ns:
                    for ts in range(TSUB):
                        emit_v_group(n, ts)
                for n in fill_ns:
                    x_tiles[n] = dma_x(n)
                for kc in range(OKC):  # not needed until the first out-proj
                    nc.sync.dma_start(
                        out=w_out_sb[:, kc, :],
                        in_=w_out_c[128 * kc:128 * (kc + 1), :],
                    )

                def qk_pair(ns, sub):
                    out = []
                    for n in ns:
                        out.append(partial(emit_qk_group, n, sub))
                        out.append(partial(emit_qk_group, n, SUBS + sub))
                    return out

                def v_tiles(ns):
                    return [partial(emit_v_group, n, ts)
                            for n in ns for ts in range(TSUB)]

                # dependency-free filler, spread round-robin
                free0 = qk_pair(fill_ns, 0) + v_tiles(fill_ns[:-1])
                if NB == 1:
                    free0 += v_tiles(fill_ns[-1:])
                pb0 = [qk_pair(head_ns, s + 1) if s + 1 < SUBS else []
                       for s in range(SUBS)]
                for idx, it in enumerate(free0):
                    pb0[idx % SUBS].append(it)
                do_block(0, pb0, attnlo)

                prev_gms = [m for m in range(LCH)]
                for jb in range(1, NB):
                    # pair s's q,k land one pair early so their DVE casts
                    # are done before pair s reads qT/kT
                    pb = [[] for _ in range(SUBS)]
                    if jb == 1:
                        for s in range(1, SUBS):
                            pb[s - 1] += qk_pair(fill_ns, s)
                        pb[0] += v_tiles(fill_ns[-1:])
                    ops = [partial(emit_outproj, g) for g in prev_gms]
                    start = 1 if SUBS > 1 else 0
                    for idx, it in enumerate(ops):
                        pb[start + idx % (SUBS - start)].append(it)
                    lastb = jb == NB - 1
                    p0g = [LCH * jb + m for m in range(PW // 128)]
                    do_block(jb, pb, attnlo,
                             self_ops=[partial(emit_outproj, g) for g in p0g]
                             if lastb else ())
                    prev_gms = [LCH * jb + m for m in range(LCH)
                                if not (lastb and LCH * jb + m in p0g)]
                for gm in prev_gms:
                    emit_outproj(gm)

    nc.finalize()
    return nc


def make_in_maps(x, w_qkv, w_out):
    x = np.asarray(x, dtype=np.float32)
    w_qkv = np.asarray(w_qkv, dtype=np.float32)
    w_out = np.asarray(w_out, dtype=np.float32)
    in_maps = []
    for core in range(N_CORES):
        b, g = core // 2, core % 2
        cols = slice(512 * g, 512 * (g + 1))
        wq = np.ascontiguousarray(
            np.concatenate(
                [w_qkv[:, cols], w_qkv[:, 1024:][:, cols], w_qkv[:, 2048:][:, cols]],
                axis=1,
            )
        )
        in_maps.append({
            "x_t": np.ascontiguousarray(x[b].T).astype(ml_dtypes.bfloat16),
            "w_qkv_c": wq.astype(ml_dtypes.bfloat16),
            "w_out_c": np.ascontiguousarray(
                w_out[512 * g:512 * (g + 1), :]).astype(ml_dtypes.bfloat16),
        })
    return in_maps


_NC_CACHE = None
LAST_RESULT = None


def kernel(x, w_qkv, w_out):
    global _NC_CACHE, LAST_RESULT
    if _NC_CACHE is None:
        _NC_CACHE = build_core_program()
    nc = _NC_CACHE
    in_maps = make_in_maps(x, w_qkv, w_out)
    res = run_bass_kernel_spmd(nc, in_maps, list(range(N_CORES)))
    LAST_RESULT = res
    outs = [r["y_part"] for r in res.results]
    y = np.stack([outs[2 * b] + outs[2 * b + 1] for b in range(B)], axis=0)
    return y.astype(np.float32)
